# revision 11
# baseline (speedup 1.0000x reference)
"""Trainium2 Bass kernel v2 for masked scaled-dot-product attention.

Key structural choices vs v1 (which was activation-engine-bound and
psum-double-buffer cycle-bound at ~293us):

  - q is processed in 512-wide chunks so a score tile [128,512] is ONE
    psum bank: 5 score slots + 3 output-accumulator slots fill the 8
    banks. Pipeline depth 5 breaks the slot-reuse cycle (reader ->
    sem -> QK -> sem -> reader, ~1.6us) that capped v1 at ~1.07us/tile.
  - ~200 of 512 score tiles skip the activation engine entirely: one
    DVE scalar_tensor_tensor computes u16 = round(s*a + mbias) where
    mbias is a per-element fp16 tensor {15296 keep, -29024 masked};
    bitcast u16 -> fp16 IS exp(s/8) (Schraudolph, ~1.8% rms), and
    masked lanes go negative -> uint16 saturates to 0 -> exact 0.
    One op fuses exp AND mask. The rest run real exp on ACT + a {0,1}
    mask multiply on DVE (fp16 2x) or on the otherwise-idle GPSIMD
    (whose PV is deferred into the chunk's back half for slack).
  - PV matmuls are issued with a 5-step lag so the PE - the 222us
    bottleneck - never head-of-line blocks on the exp chain.
  - Denominator normalize (ones-row of VX rides the PV accumulate)
    bounces through HBM for the 128-lane reciprocal + partition
    broadcast; with 3 o-slots the ~10us chain latency has 2 chunk
    periods of slack. The final chunk uses a low-latency 2-strip
    path (single-lane recip + f32r rank-1) to cut the kernel tail.

Tile classes are keyed on (q-chunk position 0..3, k-tile 0..15) so the
single per-batch mask tensor (shared by all 8 heads on a core) can bake
the right encoding per region.
"""

from contextlib import ExitStack

import numpy as np

import concourse.bass as bass
import concourse.mybir as mybir
import concourse.tile as tile
from concourse import bacc
from concourse.bass_utils import run_bass_kernel_spmd

B, H, S, D = 4, 16, 2048, 64
N_CORES = 8
HPC = (B * H) // N_CORES  # heads per core = 8
KT_TILES = S // 128  # 16
QCHUNK = 512
N_CHUNKS = S // QCHUNK  # 4
SCALE = 1.0 / np.sqrt(np.float32(D))  # 0.125

_F32 = mybir.dt.float32
_F32R = mybir.dt.float32r
_F16 = mybir.dt.float16
_U16 = mybir.dt.uint16

# fp16 Schraudolph: bitcast(uint16(round(s * 2^10*log2(e)/8 + b))) ~= exp(s/8)
A_CONST = 1477.3197 * 0.125
B_KEEP = 15296.0                      # fp16-exact bias, sigma ~= -64
B_MASK = B_KEEP - 240.0 * A_CONST     # masked: always negative -> sat to 0

LAST_EXEC_NS = None
LAST_TRACE = None

DEFAULT_CFG = dict(
    nb_per_c=(7, 7, 7, 7),    # bit-exp tiles per q-chunk position (x8 heads)
    np_per_c=(5, 4, 4, 5),    # pool-masked tiles per q-chunk position
    pv_lag=7,                 # PV issue lag (steps)
    dsb_at=1,                 # normalize stage delays after last C-PV
    recip_at=5,
    ofmul_at=13,
    tail_cols=1,              # last-chunk normalize column strips
    sbufs=6,                  # psum score slots
    obufs=2,                  # psum output-accumulator slots
)

B_POS = [2, 6, 11, 15, 8, 14, 5, 10]   # bit-exp kt positions per chunk
C_POS = [0, 4, 9, 13, 7]               # pool-mask kt positions per chunk
# (overridable via cfg["b_pos"]/cfg["c_pos"])

N_CHUNKS_TOTAL = HPC * N_CHUNKS  # 32


def tile_classes(cfg=None):
    """Class per (q-chunk position 0..3, kt): 'b' fused DVE bit-exp,
    'c' ACT exp + GPSIMD mask (deferred PV), 'a' ACT exp + DVE mask."""
    cfg = {**DEFAULT_CFG, **(cfg or {})}
    cls = {}
    b_pos = cfg.get("b_pos", B_POS)
    c_pos = cfg.get("c_pos", C_POS)
    for c in range(N_CHUNKS):
        bset = frozenset(b_pos[:cfg["nb_per_c"][c]])
        cset = frozenset(c_pos[:cfg["np_per_c"][c]])
        assert not (bset & cset)
        for kt in range(KT_TILES):
            cls[(c, kt)] = "b" if kt in bset else ("c" if kt in cset else "a")
    return cls


def build_kernel(n_heads=HPC, reps=1, cfg=None):
    cfg = {**DEFAULT_CFG, **(cfg or {})}
    LAG = cfg["pv_lag"]
    CLS = tile_classes(cfg)

    nc = bacc.Bacc(
        "TRN2", target_bir_lowering=False, debug=False, num_devices=N_CORES
    )
    n_pairs = n_heads // 2

    QT = nc.dram_tensor("qt", [n_pairs, 128, S], _F32R, kind="ExternalInput").ap()
    KT = nc.dram_tensor("kt", [n_pairs, 128, S], _F32R, kind="ExternalInput").ap()
    QT16 = nc.dram_tensor("qt16", [128, S], _F16, kind="ExternalInput").ap()
    KT16 = nc.dram_tensor("kt16", [128, S], _F16, kind="ExternalInput").ap()
    VX = nc.dram_tensor("vx", [n_heads, 128, KT_TILES * 65], _F16,
                        kind="ExternalInput").ap()
    MSK = nc.dram_tensor("msk", [KT_TILES, 128, S], _F16,
                         kind="ExternalInput").ap()
    OT = nc.dram_tensor("ot", [n_heads, 64, S], _F16, kind="ExternalOutput").ap()

    with tile.TileContext(nc) as tc, ExitStack() as ctx:
        const = ctx.enter_context(tc.tile_pool(name="const", bufs=1))
        mskp = ctx.enter_context(tc.tile_pool(name="mskp", bufs=1))
        qkp = ctx.enter_context(tc.tile_pool(name="qkp", bufs=2))
        vxp = ctx.enter_context(tc.tile_pool(name="vxp", bufs=5))
        pp = ctx.enter_context(tc.tile_pool(name="pp", bufs=16))
        pmp = ctx.enter_context(tc.tile_pool(name="pmp", bufs=16))
        pmc = ctx.enter_context(tc.tile_pool(name="pmc", bufs=8))
        ofp = ctx.enter_context(tc.tile_pool(name="ofp", bufs=3))
        rp = ctx.enter_context(tc.tile_pool(name="rp", bufs=3))
        lastp = ctx.enter_context(tc.tile_pool(name="lastp", bufs=1))
        sps = ctx.enter_context(
            tc.tile_pool(name="sps", bufs=cfg["sbufs"], space="PSUM"))
        ops = ctx.enter_context(
            tc.tile_pool(name="ops", bufs=cfg["obufs"], space="PSUM"))
        dr = ctx.enter_context(tc.tile_pool(name="dr", bufs=1, space="DRAM"))

        scr_d = dr.tile([N_CHUNKS_TOTAL, QCHUNK], _F16)
        scr_r = dr.tile([N_CHUNKS_TOTAL, QCHUNK], _F16)

        # ACT spline-table preload while first DMAs are in flight.
        warm = const.tile([1, 2], _F32)
        nc.gpsimd.memset(warm[:], 0.0)
        warm16 = const.tile([1, 2], _F16)
        nc.scalar.activation(warm16[:], warm[:],
                             mybir.ActivationFunctionType.Exp, scale=1.0)

        # ones row for the tail's rank-1 broadcast
        ones16_t = const.tile([65, 64], _F16)
        nc.gpsimd.memset(ones16_t[:], 1.0)

        mskbig = mskp.tile([128, KT_TILES * S], _F16, name="mskbig")
        msk_t = [mskbig[:, kt * S:(kt + 1) * S] for kt in range(KT_TILES)]


        # ---- deferred-PV and normalize scheduling state ----
        pend_pv = []   # (ready_step, seq, record)
        norm_q = []    # (due_step, fn)
        seq_ctr = [0]

        def push_pv(ready, rec):
            pend_pv.append((ready, seq_ctr[0], rec))
            seq_ctr[0] += 1
            pend_pv.sort(key=lambda x: (x[0], x[1]))

        def issue_pv(rec):
            o_ps, vx_sb, pm, kt, st, sp = rec
            nc.tensor.matmul(
                o_ps[:], lhsT=vx_sb[:, kt * 65:(kt + 1) * 65], rhs=pm[:],
                start=st, stop=sp,
            )

        def pump(t):
            while pend_pv and pend_pv[0][0] <= t:
                issue_pv(pend_pv.pop(0)[2])
            while norm_q and norm_q[0][0] <= t:
                norm_q.pop(0)[1]()
            norm_q.sort(key=lambda x: x[0])

        def sched_norm(ci, h, c, o_ps, E_end):
            q0 = c * QCHUNK
            d_sb = rp.tile([65, QCHUNK], _F16, tag="d")
            t128 = rp.tile([128, QCHUNK // 128], _F16, tag="t128")
            r128 = rp.tile([128, QCHUNK // 128], _F16, tag="r128")
            rb_sb = rp.tile([64, QCHUNK], _F16, tag="rb")
            of_sb = ofp.tile([64, QCHUNK], _F16, tag="of")

            def st_dsb():
                # copy ALL 65 rows (same ACT cost as 1 row: free-size bound):
                # frees the o psum bank ~12 steps earlier, so obufs=2 and a
                # 6-deep score pipeline fit in the 8 banks.
                nc.scalar.copy(d_sb[:, :], o_ps[:, :])
                # SBUF->SBUF redistribute [1,512] -> [128,4] in one DMA
                nc.sync.dma_start(t128[:], d_sb[64:65, :])

            def st_recip():
                with nc.allow_low_precision(reason="softmax denom recip"):
                    nc.vector.reciprocal(r128[:], t128[:])
                nc.sync.dma_start(
                    scr_r[ci].rearrange("(p f) -> p f", p=128), r128[:]
                )
                nc.sync.dma_start(
                    rb_sb[:], scr_r[ci].partition_broadcast(64)
                )

            def st_ofmul():
                nc.vector.tensor_mul(of_sb[:], d_sb[0:64, :], rb_sb[:])
                nc.sync.dma_start(OT[h, :, q0:q0 + QCHUNK], of_sb[:])

            norm_q.append((E_end + cfg["dsb_at"], st_dsb))
            norm_q.append((E_end + cfg["recip_at"], st_recip))
            norm_q.append((E_end + cfg["ofmul_at"], st_ofmul))
            norm_q.sort(key=lambda x: x[0])

        # ---------------- main stream ----------------
        # DMA emission runs ONE PAIR AHEAD of compute (qkp bufs=3) so the
        # SP queue's head-of-line waits (normalize bounces) can't starve
        # the next pair's QK inputs.
        loaded = {}

        def load_pair(p, first):
            if first:
                # pair 0 ships fp16 Q/K: halves the startup-critical DMA
                # bytes; PE cost identical (fp16 = 1 cycle/row like f32r).
                qt_sb = qkp.tile([128, S], _F16, tag="qt16", name="qt16t")
                kt_sb = qkp.tile([128, S], _F16, tag="kt16", name="kt16t")
            else:
                qt_sb = qkp.tile([128, S], _F32R, tag="qt", name=f"qt{p}")
                kt_sb = qkp.tile([128, S], _F32R, tag="kt", name=f"kt{p}")
            vx2 = [vxp.tile([128, KT_TILES * 65], _F16, tag="vx",
                            name=f"vx{p}_{hi}") for hi in range(2)]
            if first:
                # startup-critical order: chunk 0 needs kt block 0 and qt
                # block 0; then V and the mask tiles interleaved with the
                # remaining kt blocks in consumption order (step kt needs
                # msk_t[kt]); qt blocks 1-3 only matter from chunk 1 on.
                def mload(kt, c):
                    cs = slice(c * QCHUNK, (c + 1) * QCHUNK)
                    nc.sync.dma_start(msk_t[kt][:, cs], MSK[kt][:, cs])
                nc.sync.dma_start(kt_sb[:, 0:512], KT16[:, 0:512])
                nc.sync.dma_start(qt_sb[:, 0:512], QT16[:, 0:512])
                mload(0, 0)
                for hi in range(2):
                    nc.sync.dma_start(vx2[hi][:], VX[p * 2 + hi])
                mload(1, 0)
                mload(2, 0)
                for blk in (1, 2, 3):
                    bs = slice(blk * 512, (blk + 1) * 512)
                    nc.sync.dma_start(kt_sb[:, bs], KT16[:, bs])
                    for kt in range(blk * 4 - 1, blk * 4 + 3):
                        mload(kt, 0)
                mload(15, 0)
                nc.sync.dma_start(qt_sb[:, 512:1024], QT16[:, 512:1024])
                for kt in range(8):
                    nc.sync.dma_start(msk_t[kt][:, 512:2048],
                                      MSK[kt][:, 512:2048])
                nc.sync.dma_start(qt_sb[:, 1024:1536], QT16[:, 1024:1536])
                nc.sync.dma_start(qt_sb[:, 1536:2048], QT16[:, 1536:2048])
                for kt in range(8, KT_TILES):
                    nc.sync.dma_start(msk_t[kt][:, 512:2048],
                                      MSK[kt][:, 512:2048])
            else:
                for blk in range(4):
                    bs = slice(blk * 512, (blk + 1) * 512)
                    nc.sync.dma_start(kt_sb[:, bs], KT[p][:, bs])
                    nc.sync.dma_start(qt_sb[:, bs], QT[p][:, bs])
                for hi in range(2):
                    nc.sync.dma_start(vx2[hi][:], VX[p * 2 + hi])
            loaded[p] = (qt_sb, kt_sb, vx2)

        for rep in range(reps):
          for hp in range(n_pairs):
            if hp == 0 and hp not in loaded:
                load_pair(0, first=(rep == 0))
            if hp + 1 < n_pairs and hp + 1 not in loaded:
                load_pair(hp + 1, first=False)
            qt_sb, kt_sb, vx2 = loaded.pop(hp)
            for hi in range(2):
                h = hp * 2 + hi
                po = hi * 64
                vx_sb = vx2[hi]
                for c in range(N_CHUNKS):
                    ci = h * N_CHUNKS + c
                    q0 = c * QCHUNK
                    last_chunk = (rep == reps - 1 and ci == N_CHUNKS_TOTAL - 1)
                    o_ps = ops.tile([65, QCHUNK], _F32, tag="o")
                    S0 = ci * KT_TILES
                    # startup special-case: during the first chunks the
                    # DVE lags (masks arrive JIT); route 'a' masks to the
                    # idle GPSIMD too (same {0,1} encoding, engine is free).
                    def eff_cls(kt):
                        k = CLS[(c, kt)]
                        if ci < cfg.get("warm_chunks", 0) and k == "a":
                            return "c"
                        return k
                    ci_pool = [kt for kt in range(KT_TILES)
                               if eff_cls(kt) == "c"]
                    npc = len(ci_pool)
                    ready = {}
                    for kt in range(KT_TILES):
                        if eff_cls(kt) == "c":
                            i = ci_pool.index(kt)
                            if last_chunk:
                                ready[kt] = S0 + kt + 7
                            else:
                                ready[kt] = S0 + KT_TILES + 2 + i
                        else:
                            ready[kt] = S0 + kt + LAG
                    order = sorted(range(KT_TILES),
                                   key=lambda k: (ready[k], k))
                    start_kt, stop_kt = order[0], order[-1]
                    for kt in range(KT_TILES):
                        t = S0 + kt
                        pump(t)
                        s_ps = sps.tile([128, QCHUNK], _F32, tag="s")
                        nc.tensor.matmul(
                            s_ps[:],
                            lhsT=kt_sb[po:po + 64, kt * 128:(kt + 1) * 128],
                            rhs=qt_sb[po:po + 64, q0:q0 + QCHUNK],
                            start=True, stop=True,
                        )
                        klass = eff_cls(kt)
                        pm_sb = (pmc if klass == "c" else pmp).tile(
                            [128, QCHUNK], _F16, tag="pm")
                        if klass == "b":
                            # fused bit-exp + mask, one DVE op
                            nc.vector.scalar_tensor_tensor(
                                pm_sb[:].bitcast(_U16), s_ps[:],
                                A_CONST,
                                msk_t[kt][:, q0:q0 + QCHUNK],
                                mybir.AluOpType.mult, mybir.AluOpType.add,
                            )
                        else:
                            p_sb = pp.tile([128, QCHUNK], _F16, tag="p")
                            nc.scalar.activation(
                                p_sb[:], s_ps[:],
                                mybir.ActivationFunctionType.Exp,
                                scale=float(SCALE),
                            )
                            eng = nc.gpsimd if klass == "c" else nc.vector
                            eng.tensor_mul(
                                pm_sb[:], p_sb[:],
                                msk_t[kt][:, q0:q0 + QCHUNK],
                            )
                        push_pv(
                            ready[kt],
                            (o_ps, vx_sb, pm_sb, kt,
                             kt == start_kt, kt == stop_kt),
                        )
                    if last_chunk:
                        continue  # tail handled after the loop
                    sched_norm(ci, h, c, o_ps, S0 + KT_TILES + 2 + npc)

          while pend_pv:
              issue_pv(pend_pv.pop(0)[2])
          while norm_q:
              norm_q.pop(0)[1]()

        # ---------------- tail: last chunk, low-latency normalize ----------
        h_last = n_heads - 1
        q0 = (N_CHUNKS - 1) * QCHUNK
        d_sb = lastp.tile([65, QCHUNK], _F32, tag="dl")
        r_sb = lastp.tile([65, QCHUNK], _F16, tag="rl")
        rb_ps = ops.tile([64, QCHUNK], _F32, tag="o")
        rb_sb2 = lastp.tile([64, QCHUNK], _F16, tag="rb2")
        of_sb = ofp.tile([64, QCHUNK], _F16, tag="of")
        nc.vector.tensor_copy(d_sb[64:65, :], o_ps[64:65, :])
        with nc.allow_low_precision(reason="denom recip"):
            nc.vector.reciprocal(r_sb[64:65, :], d_sb[64:65, :])
        nc.tensor.matmul(
            rb_ps[:], lhsT=ones16_t[64:65, :], rhs=r_sb[64:65, :],
            start=True, stop=True,
        )
        nc.vector.tensor_copy(rb_sb2[:], rb_ps[:])
        for j4 in range(2):
            cs = slice(j4 * 256, (j4 + 1) * 256)
            nc.vector.tensor_mul(of_sb[:, cs], o_ps[0:64, cs], rb_sb2[:, cs])
            nc.sync.dma_start(OT[h_last, :, q0 + j4 * 256:q0 + (j4 + 1) * 256],
                              of_sb[:, cs])
    nc.compile()
    return nc


def _encode_mask(mask_qk, cls):
    """mask_qk [S_q, S_k] bool (True = masked) -> [KT_TILES,128,S] fp16,
    encoding per (q-chunk position, kt) tile class."""
    keepT = (~mask_qk).T  # [S_k, S_q]
    out = np.empty((S, S), dtype=np.float16)
    for c in range(N_CHUNKS):
        qs = slice(c * QCHUNK, (c + 1) * QCHUNK)
        for kt in range(KT_TILES):
            rows = slice(kt * 128, (kt + 1) * 128)
            kp = keepT[rows, qs]
            if cls[(c, kt)] == "b":
                out[rows, qs] = np.where(
                    kp, np.float16(B_KEEP), np.float16(B_MASK))
            else:
                out[rows, qs] = kp.astype(np.float16)
    return np.ascontiguousarray(out).reshape(KT_TILES, 128, S)


def shard_inputs(Q, K, V, mask, n_heads=HPC, cfg=None):
    """Host-side prep: per-core input dicts matching build_kernel tensors."""
    cls = tile_classes(cfg)
    f16 = np.float16
    ones = np.ones((n_heads, S, 1), np.float32)
    in_maps = []
    maskT_cache = {}
    for cc in range(N_CORES):
        b = cc // 2
        h0 = (cc % 2) * HPC
        q = Q[b, h0:h0 + n_heads]
        k = K[b, h0:h0 + n_heads]
        v = V[b, h0:h0 + n_heads]
        qt = np.ascontiguousarray(q.transpose(0, 2, 1)).reshape(
            n_heads // 2, 128, S)
        kt = np.ascontiguousarray(k.transpose(0, 2, 1)).reshape(
            n_heads // 2, 128, S)
        vx = np.ascontiguousarray(
            np.concatenate([v, ones], axis=2)
            .reshape(n_heads, KT_TILES, 128, 65)
            .transpose(0, 2, 1, 3)
        ).reshape(n_heads, 128, KT_TILES * 65).astype(f16)
        if b not in maskT_cache:
            maskT_cache[b] = _encode_mask(mask[b, 0], cls)
        in_maps.append({"qt": qt, "kt": kt, "vx": vx, "msk": maskT_cache[b],
                        "qt16": qt[0].astype(f16), "kt16": kt[0].astype(f16)})
    return in_maps


_NC_CACHE = {}


def kernel(Q, K, V, mask, trace=False):
    global LAST_EXEC_NS, LAST_TRACE
    Q = np.asarray(Q, dtype=np.float32)
    K = np.asarray(K, dtype=np.float32)
    V = np.asarray(V, dtype=np.float32)
    mask = np.asarray(mask).astype(bool)

    if "nc" not in _NC_CACHE:
        _NC_CACHE["nc"] = build_kernel()
    nc = _NC_CACHE["nc"]

    in_maps = shard_inputs(Q, K, V, mask)
    try:
        res = run_bass_kernel_spmd(
            nc, in_maps, core_ids=list(range(N_CORES)), trace=trace
        )
    except ModuleNotFoundError:
        res = run_bass_kernel_spmd(
            nc, in_maps, core_ids=list(range(N_CORES)), trace=False
        )
    LAST_EXEC_NS = res.exec_time_ns
    LAST_TRACE = res.instructions_and_trace
    out = np.empty((B, H, S, D), np.float32)
    for cc, r in enumerate(res.results):
        b = cc // 2
        h0 = (cc % 2) * HPC
        out[b, h0:h0 + HPC] = r["ot"].astype(np.float32).transpose(0, 2, 1)
    return out


# revision 12
# speedup vs baseline: 1.0019x; 1.0019x over previous
"""Trainium2 Bass kernel v2 for masked scaled-dot-product attention.

Key structural choices vs v1 (which was activation-engine-bound and
psum-double-buffer cycle-bound at ~293us):

  - q is processed in 512-wide chunks so a score tile [128,512] is ONE
    psum bank: 5 score slots + 3 output-accumulator slots fill the 8
    banks. Pipeline depth 5 breaks the slot-reuse cycle (reader ->
    sem -> QK -> sem -> reader, ~1.6us) that capped v1 at ~1.07us/tile.
  - ~200 of 512 score tiles skip the activation engine entirely: one
    DVE scalar_tensor_tensor computes u16 = round(s*a + mbias) where
    mbias is a per-element fp16 tensor {15296 keep, -29024 masked};
    bitcast u16 -> fp16 IS exp(s/8) (Schraudolph, ~1.8% rms), and
    masked lanes go negative -> uint16 saturates to 0 -> exact 0.
    One op fuses exp AND mask. The rest run real exp on ACT + a {0,1}
    mask multiply on DVE (fp16 2x) or on the otherwise-idle GPSIMD
    (whose PV is deferred into the chunk's back half for slack).
  - PV matmuls are issued with a 5-step lag so the PE - the 222us
    bottleneck - never head-of-line blocks on the exp chain.
  - Denominator normalize (ones-row of VX rides the PV accumulate)
    bounces through HBM for the 128-lane reciprocal + partition
    broadcast; with 3 o-slots the ~10us chain latency has 2 chunk
    periods of slack. The final chunk uses a low-latency 2-strip
    path (single-lane recip + f32r rank-1) to cut the kernel tail.

Tile classes are keyed on (q-chunk position 0..3, k-tile 0..15) so the
single per-batch mask tensor (shared by all 8 heads on a core) can bake
the right encoding per region.
"""

from contextlib import ExitStack

import numpy as np

import concourse.bass as bass
import concourse.mybir as mybir
import concourse.tile as tile
from concourse import bacc
from concourse.bass_utils import run_bass_kernel_spmd

B, H, S, D = 4, 16, 2048, 64
N_CORES = 8
HPC = (B * H) // N_CORES  # heads per core = 8
KT_TILES = S // 128  # 16
QCHUNK = 512
N_CHUNKS = S // QCHUNK  # 4
SCALE = 1.0 / np.sqrt(np.float32(D))  # 0.125

_F32 = mybir.dt.float32
_F32R = mybir.dt.float32r
_F16 = mybir.dt.float16
_U16 = mybir.dt.uint16

# fp16 Schraudolph: bitcast(uint16(round(s * 2^10*log2(e)/8 + b))) ~= exp(s/8)
A_CONST = 1477.3197 * 0.125
B_KEEP = 15296.0                      # fp16-exact bias, sigma ~= -64
B_MASK = B_KEEP - 240.0 * A_CONST     # masked: always negative -> sat to 0

LAST_EXEC_NS = None
LAST_TRACE = None

DEFAULT_CFG = dict(
    nb_per_c=(7, 7, 7, 7),    # bit-exp tiles per q-chunk position (x8 heads)
    np_per_c=(5, 4, 4, 5),    # pool-masked tiles per q-chunk position
    pv_lag=7,                 # PV issue lag (steps)
    dsb_at=1,                 # normalize stage delays after last C-PV
    recip_at=5,
    ofmul_at=13,
    tail_cols=1,              # last-chunk normalize column strips
    sbufs=6,                  # psum score slots
    obufs=2,                  # psum output-accumulator slots
)

B_POS = [2, 6, 11, 15, 8, 14, 5, 10]   # bit-exp kt positions per chunk
C_POS = [0, 4, 9, 13, 7]               # pool-mask kt positions per chunk
# (overridable via cfg["b_pos"]/cfg["c_pos"])

N_CHUNKS_TOTAL = HPC * N_CHUNKS  # 32


def tile_classes(cfg=None):
    """Class per (q-chunk position 0..3, kt): 'b' fused DVE bit-exp,
    'c' ACT exp + GPSIMD mask (deferred PV), 'a' ACT exp + DVE mask."""
    cfg = {**DEFAULT_CFG, **(cfg or {})}
    cls = {}
    b_pos = cfg.get("b_pos", B_POS)
    c_pos = cfg.get("c_pos", C_POS)
    for c in range(N_CHUNKS):
        bset = frozenset(b_pos[:cfg["nb_per_c"][c]])
        cset = frozenset(c_pos[:cfg["np_per_c"][c]])
        assert not (bset & cset)
        for kt in range(KT_TILES):
            cls[(c, kt)] = "b" if kt in bset else ("c" if kt in cset else "a")
    return cls


def build_kernel(n_heads=HPC, reps=1, cfg=None):
    cfg = {**DEFAULT_CFG, **(cfg or {})}
    LAG = cfg["pv_lag"]
    CLS = tile_classes(cfg)

    nc = bacc.Bacc(
        "TRN2", target_bir_lowering=False, debug=False, num_devices=N_CORES
    )
    n_pairs = n_heads // 2

    QT = nc.dram_tensor("qt", [n_pairs, 128, S], _F32R, kind="ExternalInput").ap()
    KT = nc.dram_tensor("kt", [n_pairs, 128, S], _F32R, kind="ExternalInput").ap()
    QT16 = nc.dram_tensor("qt16", [128, S], _F16, kind="ExternalInput").ap()
    KT16 = nc.dram_tensor("kt16", [128, S], _F16, kind="ExternalInput").ap()
    VX = nc.dram_tensor("vx", [n_heads, 128, KT_TILES * 65], _F16,
                        kind="ExternalInput").ap()
    MSK = nc.dram_tensor("msk", [KT_TILES, 128, S], _F16,
                         kind="ExternalInput").ap()
    OT = nc.dram_tensor("ot", [n_heads, 64, S], _F16, kind="ExternalOutput").ap()

    with tile.TileContext(nc) as tc, ExitStack() as ctx:
        const = ctx.enter_context(tc.tile_pool(name="const", bufs=1))
        mskp = ctx.enter_context(tc.tile_pool(name="mskp", bufs=1))
        qkp = ctx.enter_context(tc.tile_pool(name="qkp", bufs=2))
        vxp = ctx.enter_context(tc.tile_pool(name="vxp", bufs=5))
        pp = ctx.enter_context(tc.tile_pool(name="pp", bufs=16))
        pmp = ctx.enter_context(tc.tile_pool(name="pmp", bufs=16))
        pmc = ctx.enter_context(tc.tile_pool(name="pmc", bufs=8))
        ofp = ctx.enter_context(tc.tile_pool(name="ofp", bufs=3))
        rp = ctx.enter_context(tc.tile_pool(name="rp", bufs=3))
        lastp = ctx.enter_context(tc.tile_pool(name="lastp", bufs=1))
        sps = ctx.enter_context(
            tc.tile_pool(name="sps", bufs=cfg["sbufs"], space="PSUM"))
        ops = ctx.enter_context(
            tc.tile_pool(name="ops", bufs=cfg["obufs"], space="PSUM"))
        dr = ctx.enter_context(tc.tile_pool(name="dr", bufs=1, space="DRAM"))

        scr_d = dr.tile([N_CHUNKS_TOTAL, QCHUNK], _F16)
        scr_r = dr.tile([N_CHUNKS_TOTAL, QCHUNK], _F16)

        # ACT spline-table preload while first DMAs are in flight.
        warm = const.tile([1, 2], _F32)
        nc.gpsimd.memset(warm[:], 0.0)
        warm16 = const.tile([1, 2], _F16)
        nc.scalar.activation(warm16[:], warm[:],
                             mybir.ActivationFunctionType.Exp, scale=1.0)

        # ones row for the tail's rank-1 broadcast
        ones16_t = const.tile([65, 64], _F16)
        nc.gpsimd.memset(ones16_t[:], 1.0)

        mskbig = mskp.tile([128, KT_TILES * S], _F16, name="mskbig")
        msk_t = [mskbig[:, kt * S:(kt + 1) * S] for kt in range(KT_TILES)]


        # ---- deferred-PV and normalize scheduling state ----
        pend_pv = []   # (ready_step, seq, record)
        norm_q = []    # (due_step, fn)
        seq_ctr = [0]

        def push_pv(ready, rec):
            pend_pv.append((ready, seq_ctr[0], rec))
            seq_ctr[0] += 1
            pend_pv.sort(key=lambda x: (x[0], x[1]))

        def issue_pv(rec):
            o_ps, vx_sb, pm, kt, st, sp = rec
            nc.tensor.matmul(
                o_ps[:], lhsT=vx_sb[:, kt * 65:(kt + 1) * 65], rhs=pm[:],
                start=st, stop=sp,
            )

        def pump(t):
            while pend_pv and pend_pv[0][0] <= t:
                issue_pv(pend_pv.pop(0)[2])
            while norm_q and norm_q[0][0] <= t:
                norm_q.pop(0)[1]()
            norm_q.sort(key=lambda x: x[0])

        def sched_norm(ci, h, c, o_ps, E_end):
            q0 = c * QCHUNK
            d_sb = rp.tile([65, QCHUNK], _F16, tag="d")
            t128 = rp.tile([128, QCHUNK // 128], _F16, tag="t128")
            r128 = rp.tile([128, QCHUNK // 128], _F16, tag="r128")
            rb_sb = rp.tile([64, QCHUNK], _F16, tag="rb")
            of_sb = ofp.tile([64, QCHUNK], _F16, tag="of")

            def st_dsb():
                # copy ALL 65 rows (same ACT cost as 1 row: free-size bound):
                # frees the o psum bank ~12 steps earlier, so obufs=2 and a
                # 6-deep score pipeline fit in the 8 banks.
                nc.scalar.copy(d_sb[:, :], o_ps[:, :])
                # SBUF->SBUF redistribute [1,512] -> [128,4] in one DMA
                nc.sync.dma_start(t128[:], d_sb[64:65, :])

            def st_recip():
                with nc.allow_low_precision(reason="softmax denom recip"):
                    nc.vector.reciprocal(r128[:], t128[:])
                nc.sync.dma_start(
                    scr_r[ci].rearrange("(p f) -> p f", p=128), r128[:]
                )
                nc.sync.dma_start(
                    rb_sb[:], scr_r[ci].partition_broadcast(64)
                )

            def st_ofmul():
                nc.vector.tensor_mul(of_sb[:], d_sb[0:64, :], rb_sb[:])
                nc.sync.dma_start(OT[h, :, q0:q0 + QCHUNK], of_sb[:])

            norm_q.append((E_end + cfg["dsb_at"], st_dsb))
            norm_q.append((E_end + cfg["recip_at"], st_recip))
            norm_q.append((E_end + cfg["ofmul_at"], st_ofmul))
            norm_q.sort(key=lambda x: x[0])

        # ---------------- main stream ----------------
        # DMA emission runs ONE PAIR AHEAD of compute (qkp bufs=3) so the
        # SP queue's head-of-line waits (normalize bounces) can't starve
        # the next pair's QK inputs.
        loaded = {}

        def load_pair(p, first):
            if first:
                # pair 0 ships fp16 Q/K: halves the startup-critical DMA
                # bytes; PE cost identical (fp16 = 1 cycle/row like f32r).
                qt_sb = qkp.tile([128, S], _F16, tag="qt16", name="qt16t")
                kt_sb = qkp.tile([128, S], _F16, tag="kt16", name="kt16t")
            else:
                qt_sb = qkp.tile([128, S], _F32R, tag="qt", name=f"qt{p}")
                kt_sb = qkp.tile([128, S], _F32R, tag="kt", name=f"kt{p}")
            vx2 = [vxp.tile([128, KT_TILES * 65], _F16, tag="vx",
                            name=f"vx{p}_{hi}") for hi in range(2)]
            if first:
                # startup-critical order: chunk 0 needs kt block 0 and qt
                # block 0; then V and the mask tiles interleaved with the
                # remaining kt blocks in consumption order (step kt needs
                # msk_t[kt]); qt blocks 1-3 only matter from chunk 1 on.
                def mload(kt, c):
                    cs = slice(c * QCHUNK, (c + 1) * QCHUNK)
                    nc.sync.dma_start(msk_t[kt][:, cs], MSK[kt][:, cs])
                nc.sync.dma_start(kt_sb[:, 0:512], KT16[:, 0:512])
                nc.sync.dma_start(qt_sb[:, 0:512], QT16[:, 0:512])
                mload(0, 0)
                for hi in range(2):
                    nc.sync.dma_start(vx2[hi][:], VX[p * 2 + hi])
                mload(1, 0)
                mload(2, 0)
                for blk in (1, 2, 3):
                    bs = slice(blk * 512, (blk + 1) * 512)
                    nc.sync.dma_start(kt_sb[:, bs], KT16[:, bs])
                    for kt in range(blk * 4 - 1, blk * 4 + 3):
                        mload(kt, 0)
                mload(15, 0)
                nc.sync.dma_start(qt_sb[:, 512:1024], QT16[:, 512:1024])
                for kt in range(8):
                    nc.sync.dma_start(msk_t[kt][:, 512:2048],
                                      MSK[kt][:, 512:2048])
                nc.sync.dma_start(qt_sb[:, 1024:1536], QT16[:, 1024:1536])
                nc.sync.dma_start(qt_sb[:, 1536:2048], QT16[:, 1536:2048])
                for kt in range(8, KT_TILES):
                    nc.sync.dma_start(msk_t[kt][:, 512:2048],
                                      MSK[kt][:, 512:2048])
            else:
                for blk in range(4):
                    bs = slice(blk * 512, (blk + 1) * 512)
                    nc.sync.dma_start(kt_sb[:, bs], KT[p][:, bs])
                    nc.sync.dma_start(qt_sb[:, bs], QT[p][:, bs])
                for hi in range(2):
                    nc.sync.dma_start(vx2[hi][:], VX[p * 2 + hi])
            loaded[p] = (qt_sb, kt_sb, vx2)

        for rep in range(reps):
          for hp in range(n_pairs):
            if hp == 0 and hp not in loaded:
                load_pair(0, first=(rep == 0))
            if hp + 1 < n_pairs and hp + 1 not in loaded:
                load_pair(hp + 1, first=False)
            qt_sb, kt_sb, vx2 = loaded.pop(hp)
            for hi in range(2):
                h = hp * 2 + hi
                po = hi * 64
                vx_sb = vx2[hi]
                for c in range(N_CHUNKS):
                    ci = h * N_CHUNKS + c
                    q0 = c * QCHUNK
                    last_chunk = (rep == reps - 1 and ci == N_CHUNKS_TOTAL - 1)
                    o_ps = ops.tile([65, QCHUNK], _F32, tag="o")
                    S0 = ci * KT_TILES
                    # startup special-case: during the first chunks the
                    # DVE lags (masks arrive JIT); route 'a' masks to the
                    # idle GPSIMD too (same {0,1} encoding, engine is free).
                    def eff_cls(kt):
                        k = CLS[(c, kt)]
                        if ci < cfg.get("warm_chunks", 0) and k == "a":
                            return "c"
                        return k
                    ci_pool = [kt for kt in range(KT_TILES)
                               if eff_cls(kt) == "c"]
                    npc = len(ci_pool)
                    ready = {}
                    for kt in range(KT_TILES):
                        if eff_cls(kt) == "c":
                            i = ci_pool.index(kt)
                            if last_chunk:
                                ready[kt] = S0 + kt + 7
                            else:
                                ready[kt] = S0 + KT_TILES + 2 + i
                        else:
                            ready[kt] = S0 + kt + LAG
                    order = sorted(range(KT_TILES),
                                   key=lambda k: (ready[k], k))
                    start_kt, stop_kt = order[0], order[-1]
                    for kt in range(KT_TILES):
                        t = S0 + kt
                        pump(t)
                        s_ps = sps.tile([128, QCHUNK], _F32, tag="s")
                        nc.tensor.matmul(
                            s_ps[:],
                            lhsT=kt_sb[po:po + 64, kt * 128:(kt + 1) * 128],
                            rhs=qt_sb[po:po + 64, q0:q0 + QCHUNK],
                            start=True, stop=True,
                        )
                        klass = eff_cls(kt)
                        pm_sb = (pmc if klass == "c" else pmp).tile(
                            [128, QCHUNK], _F16, tag="pm")
                        if klass == "b":
                            # fused bit-exp + mask, one DVE op
                            nc.vector.scalar_tensor_tensor(
                                pm_sb[:].bitcast(_U16), s_ps[:],
                                A_CONST,
                                msk_t[kt][:, q0:q0 + QCHUNK],
                                mybir.AluOpType.mult, mybir.AluOpType.add,
                            )
                        else:
                            p_sb = pp.tile([128, QCHUNK], _F16, tag="p")
                            nc.scalar.activation(
                                p_sb[:], s_ps[:],
                                mybir.ActivationFunctionType.Exp,
                                scale=float(SCALE),
                            )
                            eng = nc.gpsimd if klass == "c" else nc.vector
                            eng.tensor_mul(
                                pm_sb[:], p_sb[:],
                                msk_t[kt][:, q0:q0 + QCHUNK],
                            )
                        push_pv(
                            ready[kt],
                            (o_ps, vx_sb, pm_sb, kt,
                             kt == start_kt, kt == stop_kt),
                        )
                    if last_chunk:
                        continue  # tail handled after the loop
                    sched_norm(ci, h, c, o_ps, S0 + KT_TILES + 2 + npc)

          while pend_pv:
              issue_pv(pend_pv.pop(0)[2])
          while norm_q:
              norm_q.pop(0)[1]()

        # ---------------- tail: last chunk, low-latency normalize ----------
        h_last = n_heads - 1
        q0 = (N_CHUNKS - 1) * QCHUNK
        d_sb = lastp.tile([65, QCHUNK], _F32, tag="dl")
        r_sb = lastp.tile([65, QCHUNK], _F16, tag="rl")
        rb_ps = ops.tile([64, QCHUNK], _F32, tag="o")
        rb_sb2 = lastp.tile([64, QCHUNK], _F16, tag="rb2")
        of_sb = ofp.tile([64, QCHUNK], _F16, tag="of")
        with nc.allow_low_precision(reason="denom recip"):
            nc.vector.reciprocal(r_sb[64:65, :], o_ps[64:65, :])
        nc.tensor.matmul(
            rb_ps[:], lhsT=ones16_t[64:65, :], rhs=r_sb[64:65, :],
            start=True, stop=True,
        )
        nc.vector.tensor_copy(rb_sb2[:], rb_ps[:])
        for j4 in range(2):
            cs = slice(j4 * 256, (j4 + 1) * 256)
            nc.vector.tensor_mul(of_sb[:, cs], o_ps[0:64, cs], rb_sb2[:, cs])
            nc.sync.dma_start(OT[h_last, :, q0 + j4 * 256:q0 + (j4 + 1) * 256],
                              of_sb[:, cs])
    nc.compile()
    return nc


def _encode_mask(mask_qk, cls):
    """mask_qk [S_q, S_k] bool (True = masked) -> [KT_TILES,128,S] fp16,
    encoding per (q-chunk position, kt) tile class."""
    keepT = (~mask_qk).T  # [S_k, S_q]
    out = np.empty((S, S), dtype=np.float16)
    for c in range(N_CHUNKS):
        qs = slice(c * QCHUNK, (c + 1) * QCHUNK)
        for kt in range(KT_TILES):
            rows = slice(kt * 128, (kt + 1) * 128)
            kp = keepT[rows, qs]
            if cls[(c, kt)] == "b":
                out[rows, qs] = np.where(
                    kp, np.float16(B_KEEP), np.float16(B_MASK))
            else:
                out[rows, qs] = kp.astype(np.float16)
    return np.ascontiguousarray(out).reshape(KT_TILES, 128, S)


def shard_inputs(Q, K, V, mask, n_heads=HPC, cfg=None):
    """Host-side prep: per-core input dicts matching build_kernel tensors."""
    cls = tile_classes(cfg)
    f16 = np.float16
    ones = np.ones((n_heads, S, 1), np.float32)
    in_maps = []
    maskT_cache = {}
    for cc in range(N_CORES):
        b = cc // 2
        h0 = (cc % 2) * HPC
        q = Q[b, h0:h0 + n_heads]
        k = K[b, h0:h0 + n_heads]
        v = V[b, h0:h0 + n_heads]
        qt = np.ascontiguousarray(q.transpose(0, 2, 1)).reshape(
            n_heads // 2, 128, S)
        kt = np.ascontiguousarray(k.transpose(0, 2, 1)).reshape(
            n_heads // 2, 128, S)
        vx = np.ascontiguousarray(
            np.concatenate([v, ones], axis=2)
            .reshape(n_heads, KT_TILES, 128, 65)
            .transpose(0, 2, 1, 3)
        ).reshape(n_heads, 128, KT_TILES * 65).astype(f16)
        if b not in maskT_cache:
            maskT_cache[b] = _encode_mask(mask[b, 0], cls)
        in_maps.append({"qt": qt, "kt": kt, "vx": vx, "msk": maskT_cache[b],
                        "qt16": qt[0].astype(f16), "kt16": kt[0].astype(f16)})
    return in_maps


_NC_CACHE = {}


def kernel(Q, K, V, mask, trace=False):
    global LAST_EXEC_NS, LAST_TRACE
    Q = np.asarray(Q, dtype=np.float32)
    K = np.asarray(K, dtype=np.float32)
    V = np.asarray(V, dtype=np.float32)
    mask = np.asarray(mask).astype(bool)

    if "nc" not in _NC_CACHE:
        _NC_CACHE["nc"] = build_kernel()
    nc = _NC_CACHE["nc"]

    in_maps = shard_inputs(Q, K, V, mask)
    try:
        res = run_bass_kernel_spmd(
            nc, in_maps, core_ids=list(range(N_CORES)), trace=trace
        )
    except ModuleNotFoundError:
        res = run_bass_kernel_spmd(
            nc, in_maps, core_ids=list(range(N_CORES)), trace=False
        )
    LAST_EXEC_NS = res.exec_time_ns
    LAST_TRACE = res.instructions_and_trace
    out = np.empty((B, H, S, D), np.float32)
    for cc, r in enumerate(res.results):
        b = cc // 2
        h0 = (cc % 2) * HPC
        out[b, h0:h0 + HPC] = r["ot"].astype(np.float32).transpose(0, 2, 1)
    return out


# revision 13
# speedup vs baseline: 1.0022x; 1.0003x over previous
"""Trainium2 Bass kernel v2 for masked scaled-dot-product attention.

Key structural choices vs v1 (which was activation-engine-bound and
psum-double-buffer cycle-bound at ~293us):

  - q is processed in 512-wide chunks so a score tile [128,512] is ONE
    psum bank: 5 score slots + 3 output-accumulator slots fill the 8
    banks. Pipeline depth 5 breaks the slot-reuse cycle (reader ->
    sem -> QK -> sem -> reader, ~1.6us) that capped v1 at ~1.07us/tile.
  - ~200 of 512 score tiles skip the activation engine entirely: one
    DVE scalar_tensor_tensor computes u16 = round(s*a + mbias) where
    mbias is a per-element fp16 tensor {15296 keep, -29024 masked};
    bitcast u16 -> fp16 IS exp(s/8) (Schraudolph, ~1.8% rms), and
    masked lanes go negative -> uint16 saturates to 0 -> exact 0.
    One op fuses exp AND mask. The rest run real exp on ACT + a {0,1}
    mask multiply on DVE (fp16 2x) or on the otherwise-idle GPSIMD
    (whose PV is deferred into the chunk's back half for slack).
  - PV matmuls are issued with a 5-step lag so the PE - the 222us
    bottleneck - never head-of-line blocks on the exp chain.
  - Denominator normalize (ones-row of VX rides the PV accumulate)
    bounces through HBM for the 128-lane reciprocal + partition
    broadcast; with 3 o-slots the ~10us chain latency has 2 chunk
    periods of slack. The final chunk uses a low-latency 2-strip
    path (single-lane recip + f32r rank-1) to cut the kernel tail.

Tile classes are keyed on (q-chunk position 0..3, k-tile 0..15) so the
single per-batch mask tensor (shared by all 8 heads on a core) can bake
the right encoding per region.
"""

from contextlib import ExitStack

import numpy as np

import concourse.bass as bass
import concourse.mybir as mybir
import concourse.tile as tile
from concourse import bacc
from concourse.bass_utils import run_bass_kernel_spmd

B, H, S, D = 4, 16, 2048, 64
N_CORES = 8
HPC = (B * H) // N_CORES  # heads per core = 8
KT_TILES = S // 128  # 16
QCHUNK = 512
N_CHUNKS = S // QCHUNK  # 4
SCALE = 1.0 / np.sqrt(np.float32(D))  # 0.125

_F32 = mybir.dt.float32
_F32R = mybir.dt.float32r
_F16 = mybir.dt.float16
_U16 = mybir.dt.uint16

# fp16 Schraudolph: bitcast(uint16(round(s * 2^10*log2(e)/8 + b))) ~= exp(s/8)
A_CONST = 1477.3197 * 0.125
B_KEEP = 15296.0                      # fp16-exact bias, sigma ~= -64
B_MASK = B_KEEP - 240.0 * A_CONST     # masked: always negative -> sat to 0

LAST_EXEC_NS = None
LAST_TRACE = None

DEFAULT_CFG = dict(
    nb_per_c=(7, 7, 7, 7),    # bit-exp tiles per q-chunk position (x8 heads)
    np_per_c=(5, 5, 4, 5),    # pool-masked tiles per q-chunk position
    pv_lag=7,                 # PV issue lag (steps)
    dsb_at=1,                 # normalize stage delays after last C-PV
    recip_at=5,
    ofmul_at=13,
    tail_cols=1,              # last-chunk normalize column strips
    sbufs=6,                  # psum score slots
    obufs=2,                  # psum output-accumulator slots
)

B_POS = [2, 6, 11, 15, 8, 14, 5, 10]   # bit-exp kt positions per chunk
C_POS = [0, 4, 9, 13, 7]               # pool-mask kt positions per chunk
# (overridable via cfg["b_pos"]/cfg["c_pos"])

N_CHUNKS_TOTAL = HPC * N_CHUNKS  # 32


def tile_classes(cfg=None):
    """Class per (q-chunk position 0..3, kt): 'b' fused DVE bit-exp,
    'c' ACT exp + GPSIMD mask (deferred PV), 'a' ACT exp + DVE mask."""
    cfg = {**DEFAULT_CFG, **(cfg or {})}
    cls = {}
    b_pos = cfg.get("b_pos", B_POS)
    c_pos = cfg.get("c_pos", C_POS)
    for c in range(N_CHUNKS):
        bset = frozenset(b_pos[:cfg["nb_per_c"][c]])
        cset = frozenset(c_pos[:cfg["np_per_c"][c]])
        assert not (bset & cset)
        for kt in range(KT_TILES):
            cls[(c, kt)] = "b" if kt in bset else ("c" if kt in cset else "a")
    return cls


def build_kernel(n_heads=HPC, reps=1, cfg=None):
    cfg = {**DEFAULT_CFG, **(cfg or {})}
    LAG = cfg["pv_lag"]
    CLS = tile_classes(cfg)

    nc = bacc.Bacc(
        "TRN2", target_bir_lowering=False, debug=False, num_devices=N_CORES
    )
    n_pairs = n_heads // 2

    QT = nc.dram_tensor("qt", [n_pairs, 128, S], _F32R, kind="ExternalInput").ap()
    KT = nc.dram_tensor("kt", [n_pairs, 128, S], _F32R, kind="ExternalInput").ap()
    QT16 = nc.dram_tensor("qt16", [128, S], _F16, kind="ExternalInput").ap()
    KT16 = nc.dram_tensor("kt16", [128, S], _F16, kind="ExternalInput").ap()
    VX = nc.dram_tensor("vx", [n_heads, 128, KT_TILES * 65], _F16,
                        kind="ExternalInput").ap()
    MSK = nc.dram_tensor("msk", [KT_TILES, 128, S], _F16,
                         kind="ExternalInput").ap()
    OT = nc.dram_tensor("ot", [n_heads, 64, S], _F16, kind="ExternalOutput").ap()

    with tile.TileContext(nc) as tc, ExitStack() as ctx:
        const = ctx.enter_context(tc.tile_pool(name="const", bufs=1))
        mskp = ctx.enter_context(tc.tile_pool(name="mskp", bufs=1))
        qkp = ctx.enter_context(tc.tile_pool(name="qkp", bufs=2))
        vxp = ctx.enter_context(tc.tile_pool(name="vxp", bufs=5))
        pp = ctx.enter_context(tc.tile_pool(name="pp", bufs=16))
        pmp = ctx.enter_context(tc.tile_pool(name="pmp", bufs=16))
        pmc = ctx.enter_context(tc.tile_pool(name="pmc", bufs=8))
        ofp = ctx.enter_context(tc.tile_pool(name="ofp", bufs=3))
        rp = ctx.enter_context(tc.tile_pool(name="rp", bufs=3))
        lastp = ctx.enter_context(tc.tile_pool(name="lastp", bufs=1))
        sps = ctx.enter_context(
            tc.tile_pool(name="sps", bufs=cfg["sbufs"], space="PSUM"))
        ops = ctx.enter_context(
            tc.tile_pool(name="ops", bufs=cfg["obufs"], space="PSUM"))
        dr = ctx.enter_context(tc.tile_pool(name="dr", bufs=1, space="DRAM"))

        scr_d = dr.tile([N_CHUNKS_TOTAL, QCHUNK], _F16)
        scr_r = dr.tile([N_CHUNKS_TOTAL, QCHUNK], _F16)

        # ACT spline-table preload while first DMAs are in flight.
        warm = const.tile([1, 2], _F32)
        nc.gpsimd.memset(warm[:], 0.0)
        warm16 = const.tile([1, 2], _F16)
        nc.scalar.activation(warm16[:], warm[:],
                             mybir.ActivationFunctionType.Exp, scale=1.0)

        # ones row for the tail's rank-1 broadcast
        ones16_t = const.tile([65, 64], _F16)
        nc.gpsimd.memset(ones16_t[:], 1.0)

        mskbig = mskp.tile([128, KT_TILES * S], _F16, name="mskbig")
        msk_t = [mskbig[:, kt * S:(kt + 1) * S] for kt in range(KT_TILES)]


        # ---- deferred-PV and normalize scheduling state ----
        pend_pv = []   # (ready_step, seq, record)
        norm_q = []    # (due_step, fn)
        seq_ctr = [0]

        def push_pv(ready, rec):
            pend_pv.append((ready, seq_ctr[0], rec))
            seq_ctr[0] += 1
            pend_pv.sort(key=lambda x: (x[0], x[1]))

        def issue_pv(rec):
            o_ps, vx_sb, pm, kt, st, sp = rec
            nc.tensor.matmul(
                o_ps[:], lhsT=vx_sb[:, kt * 65:(kt + 1) * 65], rhs=pm[:],
                start=st, stop=sp,
            )

        def pump(t):
            while pend_pv and pend_pv[0][0] <= t:
                issue_pv(pend_pv.pop(0)[2])
            while norm_q and norm_q[0][0] <= t:
                norm_q.pop(0)[1]()
            norm_q.sort(key=lambda x: x[0])

        def sched_norm(ci, h, c, o_ps, E_end):
            q0 = c * QCHUNK
            d_sb = rp.tile([65, QCHUNK], _F16, tag="d")
            t128 = rp.tile([128, QCHUNK // 128], _F16, tag="t128")
            r128 = rp.tile([128, QCHUNK // 128], _F16, tag="r128")
            rb_sb = rp.tile([64, QCHUNK], _F16, tag="rb")
            of_sb = ofp.tile([64, QCHUNK], _F16, tag="of")

            def st_dsb():
                # copy ALL 65 rows (same ACT cost as 1 row: free-size bound):
                # frees the o psum bank ~12 steps earlier, so obufs=2 and a
                # 6-deep score pipeline fit in the 8 banks.
                nc.scalar.copy(d_sb[:, :], o_ps[:, :])
                # SBUF->SBUF redistribute [1,512] -> [128,4] in one DMA
                nc.sync.dma_start(t128[:], d_sb[64:65, :])

            def st_recip():
                with nc.allow_low_precision(reason="softmax denom recip"):
                    nc.vector.reciprocal(r128[:], t128[:])
                nc.sync.dma_start(
                    scr_r[ci].rearrange("(p f) -> p f", p=128), r128[:]
                )
                nc.sync.dma_start(
                    rb_sb[:], scr_r[ci].partition_broadcast(64)
                )

            def st_ofmul():
                nc.vector.tensor_mul(of_sb[:], d_sb[0:64, :], rb_sb[:])
                nc.sync.dma_start(OT[h, :, q0:q0 + QCHUNK], of_sb[:])

            norm_q.append((E_end + cfg["dsb_at"], st_dsb))
            norm_q.append((E_end + cfg["recip_at"], st_recip))
            norm_q.append((E_end + cfg["ofmul_at"], st_ofmul))
            norm_q.sort(key=lambda x: x[0])

        # ---------------- main stream ----------------
        # DMA emission runs ONE PAIR AHEAD of compute (qkp bufs=3) so the
        # SP queue's head-of-line waits (normalize bounces) can't starve
        # the next pair's QK inputs.
        loaded = {}

        def load_pair(p, first):
            if first:
                # pair 0 ships fp16 Q/K: halves the startup-critical DMA
                # bytes; PE cost identical (fp16 = 1 cycle/row like f32r).
                qt_sb = qkp.tile([128, S], _F16, tag="qt16", name="qt16t")
                kt_sb = qkp.tile([128, S], _F16, tag="kt16", name="kt16t")
            else:
                qt_sb = qkp.tile([128, S], _F32R, tag="qt", name=f"qt{p}")
                kt_sb = qkp.tile([128, S], _F32R, tag="kt", name=f"kt{p}")
            vx2 = [vxp.tile([128, KT_TILES * 65], _F16, tag="vx",
                            name=f"vx{p}_{hi}") for hi in range(2)]
            if first:
                # startup-critical order: chunk 0 needs kt block 0 and qt
                # block 0; then V and the mask tiles interleaved with the
                # remaining kt blocks in consumption order (step kt needs
                # msk_t[kt]); qt blocks 1-3 only matter from chunk 1 on.
                def mload(kt, c):
                    cs = slice(c * QCHUNK, (c + 1) * QCHUNK)
                    nc.sync.dma_start(msk_t[kt][:, cs], MSK[kt][:, cs])
                nc.sync.dma_start(kt_sb[:, 0:512], KT16[:, 0:512])
                nc.sync.dma_start(qt_sb[:, 0:512], QT16[:, 0:512])
                mload(0, 0)
                for hi in range(2):
                    nc.sync.dma_start(vx2[hi][:], VX[p * 2 + hi])
                mload(1, 0)
                mload(2, 0)
                for blk in (1, 2, 3):
                    bs = slice(blk * 512, (blk + 1) * 512)
                    nc.sync.dma_start(kt_sb[:, bs], KT16[:, bs])
                    for kt in range(blk * 4 - 1, blk * 4 + 3):
                        mload(kt, 0)
                mload(15, 0)
                nc.sync.dma_start(qt_sb[:, 512:1024], QT16[:, 512:1024])
                for kt in range(8):
                    nc.sync.dma_start(msk_t[kt][:, 512:2048],
                                      MSK[kt][:, 512:2048])
                nc.sync.dma_start(qt_sb[:, 1024:1536], QT16[:, 1024:1536])
                nc.sync.dma_start(qt_sb[:, 1536:2048], QT16[:, 1536:2048])
                for kt in range(8, KT_TILES):
                    nc.sync.dma_start(msk_t[kt][:, 512:2048],
                                      MSK[kt][:, 512:2048])
            else:
                for blk in range(4):
                    bs = slice(blk * 512, (blk + 1) * 512)
                    nc.sync.dma_start(kt_sb[:, bs], KT[p][:, bs])
                    nc.sync.dma_start(qt_sb[:, bs], QT[p][:, bs])
                for hi in range(2):
                    nc.sync.dma_start(vx2[hi][:], VX[p * 2 + hi])
            loaded[p] = (qt_sb, kt_sb, vx2)

        for rep in range(reps):
          for hp in range(n_pairs):
            if hp == 0 and hp not in loaded:
                load_pair(0, first=(rep == 0))
            if hp + 1 < n_pairs and hp + 1 not in loaded:
                load_pair(hp + 1, first=False)
            qt_sb, kt_sb, vx2 = loaded.pop(hp)
            for hi in range(2):
                h = hp * 2 + hi
                po = hi * 64
                vx_sb = vx2[hi]
                for c in range(N_CHUNKS):
                    ci = h * N_CHUNKS + c
                    q0 = c * QCHUNK
                    last_chunk = (rep == reps - 1 and ci == N_CHUNKS_TOTAL - 1)
                    o_ps = ops.tile([65, QCHUNK], _F32, tag="o")
                    S0 = ci * KT_TILES
                    # startup special-case: during the first chunks the
                    # DVE lags (masks arrive JIT); route 'a' masks to the
                    # idle GPSIMD too (same {0,1} encoding, engine is free).
                    def eff_cls(kt):
                        k = CLS[(c, kt)]
                        if ci < cfg.get("warm_chunks", 0) and k == "a":
                            return "c"
                        return k
                    ci_pool = [kt for kt in range(KT_TILES)
                               if eff_cls(kt) == "c"]
                    npc = len(ci_pool)
                    ready = {}
                    for kt in range(KT_TILES):
                        if eff_cls(kt) == "c":
                            i = ci_pool.index(kt)
                            if last_chunk:
                                ready[kt] = S0 + kt + 7
                            else:
                                ready[kt] = S0 + KT_TILES + 2 + i
                        else:
                            ready[kt] = S0 + kt + LAG
                    order = sorted(range(KT_TILES),
                                   key=lambda k: (ready[k], k))
                    start_kt, stop_kt = order[0], order[-1]
                    for kt in range(KT_TILES):
                        t = S0 + kt
                        pump(t)
                        s_ps = sps.tile([128, QCHUNK], _F32, tag="s")
                        nc.tensor.matmul(
                            s_ps[:],
                            lhsT=kt_sb[po:po + 64, kt * 128:(kt + 1) * 128],
                            rhs=qt_sb[po:po + 64, q0:q0 + QCHUNK],
                            start=True, stop=True,
                        )
                        klass = eff_cls(kt)
                        pm_sb = (pmc if klass == "c" else pmp).tile(
                            [128, QCHUNK], _F16, tag="pm")
                        if klass == "b":
                            # fused bit-exp + mask, one DVE op
                            nc.vector.scalar_tensor_tensor(
                                pm_sb[:].bitcast(_U16), s_ps[:],
                                A_CONST,
                                msk_t[kt][:, q0:q0 + QCHUNK],
                                mybir.AluOpType.mult, mybir.AluOpType.add,
                            )
                        else:
                            p_sb = pp.tile([128, QCHUNK], _F16, tag="p")
                            nc.scalar.activation(
                                p_sb[:], s_ps[:],
                                mybir.ActivationFunctionType.Exp,
                                scale=float(SCALE),
                            )
                            eng = nc.gpsimd if klass == "c" else nc.vector
                            eng.tensor_mul(
                                pm_sb[:], p_sb[:],
                                msk_t[kt][:, q0:q0 + QCHUNK],
                            )
                        push_pv(
                            ready[kt],
                            (o_ps, vx_sb, pm_sb, kt,
                             kt == start_kt, kt == stop_kt),
                        )
                    if last_chunk:
                        continue  # tail handled after the loop
                    sched_norm(ci, h, c, o_ps, S0 + KT_TILES + 2 + npc)

          while pend_pv:
              issue_pv(pend_pv.pop(0)[2])
          while norm_q:
              norm_q.pop(0)[1]()

        # ---------------- tail: last chunk, low-latency normalize ----------
        h_last = n_heads - 1
        q0 = (N_CHUNKS - 1) * QCHUNK
        d_sb = lastp.tile([65, QCHUNK], _F32, tag="dl")
        r_sb = lastp.tile([65, QCHUNK], _F16, tag="rl")
        rb_ps = ops.tile([64, QCHUNK], _F32, tag="o")
        rb_sb2 = lastp.tile([64, QCHUNK], _F16, tag="rb2")
        of_sb = ofp.tile([64, QCHUNK], _F16, tag="of")
        with nc.allow_low_precision(reason="denom recip"):
            nc.vector.reciprocal(r_sb[64:65, :], o_ps[64:65, :])
        nc.tensor.matmul(
            rb_ps[:], lhsT=ones16_t[64:65, :], rhs=r_sb[64:65, :],
            start=True, stop=True,
        )
        nc.vector.tensor_copy(rb_sb2[:], rb_ps[:])
        for j4 in range(2):
            cs = slice(j4 * 256, (j4 + 1) * 256)
            nc.vector.tensor_mul(of_sb[:, cs], o_ps[0:64, cs], rb_sb2[:, cs])
            nc.sync.dma_start(OT[h_last, :, q0 + j4 * 256:q0 + (j4 + 1) * 256],
                              of_sb[:, cs])
    nc.compile()
    return nc


def _encode_mask(mask_qk, cls):
    """mask_qk [S_q, S_k] bool (True = masked) -> [KT_TILES,128,S] fp16,
    encoding per (q-chunk position, kt) tile class."""
    keepT = (~mask_qk).T  # [S_k, S_q]
    out = np.empty((S, S), dtype=np.float16)
    for c in range(N_CHUNKS):
        qs = slice(c * QCHUNK, (c + 1) * QCHUNK)
        for kt in range(KT_TILES):
            rows = slice(kt * 128, (kt + 1) * 128)
            kp = keepT[rows, qs]
            if cls[(c, kt)] == "b":
                out[rows, qs] = np.where(
                    kp, np.float16(B_KEEP), np.float16(B_MASK))
            else:
                out[rows, qs] = kp.astype(np.float16)
    return np.ascontiguousarray(out).reshape(KT_TILES, 128, S)


def shard_inputs(Q, K, V, mask, n_heads=HPC, cfg=None):
    """Host-side prep: per-core input dicts matching build_kernel tensors."""
    cls = tile_classes(cfg)
    f16 = np.float16
    ones = np.ones((n_heads, S, 1), np.float32)
    in_maps = []
    maskT_cache = {}
    for cc in range(N_CORES):
        b = cc // 2
        h0 = (cc % 2) * HPC
        q = Q[b, h0:h0 + n_heads]
        k = K[b, h0:h0 + n_heads]
        v = V[b, h0:h0 + n_heads]
        qt = np.ascontiguousarray(q.transpose(0, 2, 1)).reshape(
            n_heads // 2, 128, S)
        kt = np.ascontiguousarray(k.transpose(0, 2, 1)).reshape(
            n_heads // 2, 128, S)
        vx = np.ascontiguousarray(
            np.concatenate([v, ones], axis=2)
            .reshape(n_heads, KT_TILES, 128, 65)
            .transpose(0, 2, 1, 3)
        ).reshape(n_heads, 128, KT_TILES * 65).astype(f16)
        if b not in maskT_cache:
            maskT_cache[b] = _encode_mask(mask[b, 0], cls)
        in_maps.append({"qt": qt, "kt": kt, "vx": vx, "msk": maskT_cache[b],
                        "qt16": qt[0].astype(f16), "kt16": kt[0].astype(f16)})
    return in_maps


_NC_CACHE = {}


def kernel(Q, K, V, mask, trace=False):
    global LAST_EXEC_NS, LAST_TRACE
    Q = np.asarray(Q, dtype=np.float32)
    K = np.asarray(K, dtype=np.float32)
    V = np.asarray(V, dtype=np.float32)
    mask = np.asarray(mask).astype(bool)

    if "nc" not in _NC_CACHE:
        _NC_CACHE["nc"] = build_kernel()
    nc = _NC_CACHE["nc"]

    in_maps = shard_inputs(Q, K, V, mask)
    try:
        res = run_bass_kernel_spmd(
            nc, in_maps, core_ids=list(range(N_CORES)), trace=trace
        )
    except ModuleNotFoundError:
        res = run_bass_kernel_spmd(
            nc, in_maps, core_ids=list(range(N_CORES)), trace=False
        )
    LAST_EXEC_NS = res.exec_time_ns
    LAST_TRACE = res.instructions_and_trace
    out = np.empty((B, H, S, D), np.float32)
    for cc, r in enumerate(res.results):
        b = cc // 2
        h0 = (cc % 2) * HPC
        out[b, h0:h0 + HPC] = r["ot"].astype(np.float32).transpose(0, 2, 1)
    return out
